# revision 7
# baseline (speedup 1.0000x reference)
"""Trainium2 Bass kernel for nn_Cace_74569222193773 (CACE GNN message passing).

8-core SPMD, recv-partitioned graph (atoms in 64 edge-balanced windows of 32,
8 windows/core; edges live with their receiver window, 2x128-edge blocks per
window), AllGather halo of sender-node features, replicated weights.

Performance design (vs fp32 per-window baseline, ~2.4x faster):
  * All PE matmuls in fp16 (1 cyc/col vs fp32's 2 half-speed passes), fp32
    PSUM accumulation; A_ar path in fp8e4.  numpy-simulated end-to-end
    rel err ~6e-3 vs the 2e-2 gate.
  * Node features in a d-major layout [p=(x4,n32), (g2, per-degree blocks of
    (w8, l-in-d, c9))] so the radial / memory / chi transforms run as
    window-merged wide matmuls with flat contiguous rhs (20 MMs instead of
    128 per transform) and all psum->SBUF copies are contiguous.
  * AllGather table in fp8e4 (1440B A row + 18B fp16 chi via bitcast),
    Shared-space output: ~3MB on the wire instead of 12MB fp32.
  * psMem matmuls and the B0-half output DMA overlap the AllGather window.

kernel() takes FULL unsharded inputs, returns FULL [2000,8,5,9,2] fp32.
"""
import heapq
from math import factorial

import numpy as np

# ---- static problem config (mirrors the reference) ----
MAX_L = 3; N_RBF = 8; RB = 8; K = 3
CUTOFF = 5.5
N_NODES = 2000
MP_NORM = 1.0 / np.sqrt(25.0)
C = K * K                      # 9
NB = 1 + (MAX_L + 1)           # 5

def _lxlylz(max_l):
    out = []
    for l in range(max_l + 1):
        for lx in range(l, -1, -1):
            for ly in range(l - lx, -1, -1):
                out.append((lx, ly, l - lx - ly))
    return out

L_LIST = _lxlylz(MAX_L); NL = len(L_LIST)                       # 20
LX = np.array([t[0] for t in L_LIST]); LY = np.array([t[1] for t in L_LIST])
LZ = np.array([t[2] for t in L_LIST]); DEGS = LX + LY + LZ
MULTI = np.array([factorial(int(d)) / (factorial(int(a)) * factorial(int(b)) * factorial(int(c)))
                  for a, b, c, d in zip(LX, LY, LZ, DEGS)], dtype=np.float32)
GRP_SLICES = []                 # (l_start, l_count) per degree; DEGS is sorted
for d in range(MAX_L + 1):
    idx = np.where(DEGS == d)[0]
    GRP_SLICES.append((int(idx[0]), int(len(idx))))

# ---- sharding geometry ----
N_CORES = 8
WIN = 32                        # nodes per window
NWINC = 8                       # windows per core
NWIN = N_CORES * NWINC          # 64
NSLOT = NWIN * WIN              # 2048 node slots
EBLK = 128                      # edges per block (partition dim)
NBW = 2                         # blocks per window
NBLK = NWINC * NBW              # 16 blocks per core
EPAD = NBLK * EBLK              # 2048 edge slots per core
LC = NL * C                     # 180
GLC = 2 * LC                    # 360 = per-window feature block (g, l, c)
TW = RB * LC + 32               # fp8 table row: 1440 A bytes + 18B fp16 chi + pad

# d-major feature layout: g-block (1440) = concat over degree d of a block
# ordered (w8, l in d, c9).  DOFF[d] = elem offset of block d inside a g-block.
DOFF = [0, 72, 288, 720]
# psum tiles for the window-merged radial transforms: flat contiguous slices
# (tile_width, [(d, src_off_in_gblock, width, psum_off), ...]); psum offsets
# keep each matmul slice inside one 2KB bank.
RAD_TILES = [
    (288,  [(0, 0, 72, 0), (1, 72, 216, 72)]),
    (432,  [(2, 288, 432, 0)]),
    (1024, [(3, 720, 360, 0), (3, 1080, 360, 512)]),
]

_PCUT = np.pi / CUTOFF
_RSCL = np.sqrt(2.0 / CUTOFF)

# packed-input column layout: per-core fp32 fields then replicated consts
F32_FIELDS = ['exyz_s', 'exyz_r', 'eemb_s', 'eemb_r', 'enloc']
CONST_FIELDS = ['iota32', 'multi_l', 'war_mp', 'nvec']
F16_FIELDS = ['wbd_rad', 'wbd_radmp', 'wbd_mem', 'wbd_chi']
FIELD_W = dict(exyz_s=3 * NBLK, exyz_r=3 * NBLK, eemb_s=3 * NBLK, eemb_r=3 * NBLK,
               enloc=NBLK, iota32=WIN, multi_l=NL, war_mp=64, nvec=N_RBF,
               wbd_rad=2048, wbd_radmp=2048, wbd_mem=2048, wbd_chi=2 * NB * WIN)
FIELD_OFF = {}
_o = 0
for _f in F32_FIELDS + CONST_FIELDS:
    FIELD_OFF[_f] = _o
    _o += FIELD_W[_f]
TOTW = _o
_o = 0
for _f in F16_FIELDS:
    FIELD_OFF[_f] = _o
    _o += FIELD_W[_f]
TOTW16 = _o


def _host_prep(inputs):
    pos = np.asarray(inputs['positions'], np.float32)
    shifts = np.asarray(inputs['shifts'], np.float32)
    W_embed = np.asarray(inputs['W_embed'], np.float32)
    species = np.asarray(inputs['species'])
    ei = np.asarray(inputs['edge_index'])
    send, recv = ei[0], ei[1]

    vec = (pos[recv] + shifts - pos[send]).astype(np.float64)
    r = np.sqrt((vec * vec).sum(-1))
    keep = np.where(r < CUTOFF)[0]
    deg = np.bincount(recv[keep], minlength=N_NODES)

    # balanced node->window assignment (<=WIN nodes, balance edge load)
    order = np.argsort(-deg, kind='stable')
    win_cnt = np.zeros(NWIN, np.int64); win_load = np.zeros(NWIN, np.int64)
    win_of_node = np.zeros(N_NODES, np.int64); pos_in_win = np.zeros(N_NODES, np.int64)
    heap = [(0, w) for w in range(NWIN)]
    heapq.heapify(heap)
    for nd in order:
        popped = []
        while True:
            load, w = heapq.heappop(heap)
            if win_cnt[w] < WIN:
                break
            popped.append((load, w))
        for it in popped:
            heapq.heappush(heap, it)
        win_of_node[nd] = w; pos_in_win[nd] = win_cnt[w]
        win_cnt[w] += 1; win_load[w] += deg[nd]
        heapq.heappush(heap, (win_load[w], w))
    if win_load.max() > NBW * EBLK:
        raise RuntimeError(f"window overflow: {win_load.max()} > {NBW * EBLK}")

    slot_of_node = win_of_node * WIN + pos_in_win
    emb = W_embed[species]                       # [N, K]

    win_edges = [[] for _ in range(NWIN)]
    rk = recv[keep]
    for i, e in enumerate(keep):
        win_edges[win_of_node[rk[i]]].append(e)

    cores = []
    for ci in range(N_CORES):
        e_xyz_s = np.zeros((EPAD, 3), np.float32)
        e_xyz_r = np.zeros((EPAD, 3), np.float32)
        e_emb_s = np.zeros((EPAD, K), np.float32)
        e_emb_r = np.zeros((EPAD, K), np.float32)
        e_nloc = np.full((EPAD,), -1.0, np.float32)
        e_srow = np.zeros((EPAD,), np.int32)
        e_xyz_r[:, 0] = 1.0                      # pads: r = 1, finite math
        for wl in range(NWINC):
            w = ci * NWINC + wl
            eids = np.array(win_edges[w], dtype=np.int64)
            base = wl * NBW * EBLK
            cnt = len(eids)
            if cnt:
                sl = slice(base, base + cnt)
                e_xyz_s[sl] = pos[send[eids]]
                e_xyz_r[sl] = pos[recv[eids]] + shifts[eids]
                e_emb_s[sl] = emb[send[eids]]
                e_emb_r[sl] = emb[recv[eids]]
                e_nloc[sl] = pos_in_win[recv[eids]].astype(np.float32)
                e_srow[sl] = slot_of_node[send[eids]].astype(np.int32)

        # device layout: edge e = blk*128 + p  ->  [128, NBLK(, d)]
        def dev(x):
            if x.ndim == 1:
                return np.ascontiguousarray(x.reshape(NBLK, EBLK).T)
            return np.ascontiguousarray(np.transpose(x.reshape(NBLK, EBLK, -1), (1, 0, 2)))

        # axis-major planes [128, 3*NBLK] = [a, blk]
        def axmajor(x3):
            d = dev(x3)                                  # [128, NBLK, 3]
            return np.ascontiguousarray(np.transpose(d, (0, 2, 1)).reshape(EBLK, 3 * NBLK))

        cores.append(dict(
            exyz_s=axmajor(e_xyz_s), exyz_r=axmajor(e_xyz_r),
            eemb_s=axmajor(e_emb_s), eemb_r=axmajor(e_emb_r),
            enloc=np.ascontiguousarray(dev(e_nloc)),
            esrow=np.ascontiguousarray(dev(e_srow)),
            raw=dict(e_xyz_s=e_xyz_s.copy(), e_xyz_r=e_xyz_r.copy(),
                     e_emb_s=e_emb_s.copy(), e_emb_r=e_emb_r.copy(),
                     e_nloc=e_nloc.copy(), e_srow=e_srow.copy()),
        ))

    Wr = np.asarray(inputs['W_radial'], np.float32)   # [4(deg), 8(r), 8(b)]
    Wm = np.asarray(inputs['W_mem'], np.float32)
    Wc = np.asarray(inputs['W_chi'], np.float32)      # [8(b), 5(k)]
    Wa = np.asarray(inputs['W_ar'], np.float32)       # [8(r), 8(b)]
    I32 = np.eye(WIN, dtype=np.float32)

    def bd(W):
        # [4,8,8] -> [128, (gout,d,gin)*128]: kron(W[d, gin*4:+4, gout*4:+4], I32)
        cols = []
        for gout in range(2):
            for d in range(4):
                for gin in range(2):
                    cols.append(np.kron(W[d, gin * 4:gin * 4 + 4, gout * 4:gout * 4 + 4], I32))
        return np.concatenate(cols, axis=1)          # [128, 2048]

    wchi_cols = []
    for g in range(2):
        for k in range(NB):
            wchi_cols.append(np.kron(Wc[g * 4:g * 4 + 4, k:k + 1], I32))   # [128, 32]
    consts32 = dict(
        multi_l=np.tile(MULTI.reshape(1, NL), (EBLK, 1)),            # [128, 20]
        iota32=np.tile(np.arange(WIN, dtype=np.float32).reshape(1, WIN), (EBLK, 1)),
        war_mp=np.tile((Wa * MP_NORM).reshape(1, 64), (EBLK, 1)),    # [128, 64] (r-major)
        nvec=np.tile((np.arange(1, N_RBF + 1, dtype=np.float32) / CUTOFF).reshape(1, N_RBF),
                     (EBLK, 1)),
    )
    consts16 = dict(
        wbd_rad=bd(Wr),
        wbd_radmp=bd(Wr * MP_NORM),
        wbd_mem=bd(Wm),
        wbd_chi=np.concatenate(wchi_cols, axis=1),                   # [128, 320]
    )
    edata16 = np.ascontiguousarray(
        np.concatenate([consts16[nm] for nm in F16_FIELDS], axis=1), np.float16)
    packed = []
    for ci in range(N_CORES):
        cols = [cores[ci][nm] for nm in F32_FIELDS]
        cols += [consts32[nm] for nm in CONST_FIELDS]
        packed.append(dict(edata=np.ascontiguousarray(np.concatenate(cols, axis=1), np.float32),
                           edata16=edata16,
                           esrow=cores[ci]['esrow']))
    _host_prep.aux = dict(cores=cores, slot_of_node=slot_of_node)
    return packed, slot_of_node


def _build_program(debug=False):
    import concourse.bass as bass
    import concourse.mybir as mybir
    from concourse import bacc
    from concourse.tile import TileContext

    F32 = mybir.dt.float32
    F16 = mybir.dt.float16
    AF = mybir.ActivationFunctionType
    OP = mybir.AluOpType

    nc = bacc.Bacc("TRN2", target_bir_lowering=False, debug=False,
                   num_devices=N_CORES)

    edata = nc.dram_tensor('edata', [EBLK, TOTW], F32, kind="ExternalInput")
    edata16_d = nc.dram_tensor('edata16', [EBLK, TOTW16], F16, kind="ExternalInput")
    esrow_d = nc.dram_tensor('esrow', [EBLK, NBLK], mybir.dt.int32, kind="ExternalInput")
    if debug:
        dbg = {nm: nc.dram_tensor('dbg_' + nm, sh, dt, kind="ExternalOutput")
               for nm, sh, dt in [
                   ('A_s', [EBLK, 2880], F16),
                   ('A0s', [EBLK, 2880], F16),
                   ('Anew', [EBLK, 2880], F32),
                   ('chiS', [WIN, NWINC * C], F16),
                   ('B0s', [EBLK, 2 * NB * NWINC * C], F32),
                   ('ag0', [EBLK, TW], mybir.dt.float8e4),
                   ('P', [EBLK, NBLK * LC], F16),
                   ('Ab0', [EBLK, 2880], F16),
                   ('Ar', [EBLK, 2880], F32),
                   ('radf', [EBLK, N_RBF * NBLK], F32),
                   ('fr', [EBLK, NBLK * N_RBF], F32),
                   ('Tloc', [NWINC * WIN, TW], mybir.dt.float8e4)]}
    outB = nc.dram_tensor('outB', [EBLK, 2 * 2 * NB * NWINC * C], F32,
                          kind="ExternalOutput")

    with TileContext(nc) as tc:
        with (tc.tile_pool(name="const", bufs=1) as cp,
              tc.tile_pool(name="work", bufs=4) as wp,
              tc.tile_pool(name="gat", bufs=4) as gp,
              tc.tile_pool(name="psum", bufs=2, space="PSUM") as pp,
              tc.tile_pool(name="psrad", bufs=1, space="PSUM") as pr,
              tc.tile_pool(name="dram", bufs=1, space="DRAM") as dp):

            big = cp.tile([EBLK, TOTW], F32, name='big', tag='big')
            nc.sync.dma_start(out=big[:], in_=edata[:])
            big16 = cp.tile([EBLK, TOTW16], F16, name='big16', tag='big16')
            nc.sync.dma_start(out=big16[:], in_=edata16_d[:])
            esrow_s = cp.tile([EBLK, NBLK], mybir.dt.int32, name='esrow_s', tag='esrow_s')
            nc.sync.dma_start(out=esrow_s[:], in_=esrow_d[:])

            # tiny warm-up AllGather: absorbs first-collective setup latency
            # on the idle CC queue while edge prep runs
            warm_l = dp.tile([8, 16], F32, name='warm_l')
            warm_f = dp.tile([64, 16], F32, name='warm_f', addr_space="Shared")
            nc.sync.dma_start(out=warm_l[:], in_=big[0:8, 0:16])
            nc.gpsimd.collective_compute(
                "AllGather", mybir.AluOpType.bypass,
                replica_groups=[list(range(N_CORES))],
                ins=[warm_l[:].opt()], outs=[warm_f[:].opt()])

            class _S:
                def __init__(self, t):
                    self.t = t
                def __getitem__(self, nm):
                    off = FIELD_OFF[nm]
                    return self.t[:, off:off + FIELD_W[nm]]
            s = _S(big)
            s16 = _S(big16)

            def ctile(tag, shape, dtype=F32):
                return cp.tile(shape, dtype, name=tag, tag=tag)

            TT = nc.vector.tensor_tensor
            TS = nc.vector.tensor_scalar

            # ---- geometry, edge-major [128, a*NBLK+blk] ----
            vd = ctile('vd', [EBLK, 3 * NBLK])
            TT(out=vd[:], in0=s['exyz_r'][:], in1=s['exyz_s'][:], op=OP.subtract)
            sq = ctile('sq', [EBLK, 3 * NBLK])
            TT(out=sq[:], in0=vd[:], in1=vd[:], op=OP.mult)
            r2 = ctile('r2', [EBLK, NBLK])
            TT(out=r2[:], in0=sq[:, 0:NBLK], in1=sq[:, NBLK:2 * NBLK], op=OP.add)
            TT(out=r2[:], in0=r2[:], in1=sq[:, 2 * NBLK:3 * NBLK], op=OP.add)
            rr = ctile('rr', [EBLK, NBLK])
            nc.scalar.activation(out=rr[:], in_=r2[:], func=AF.Sqrt)
            rpe = ctile('rpe', [EBLK, NBLK])
            TS(out=rpe[:], in0=rr[:], scalar1=1e-9, scalar2=None, op0=OP.add)
            rinv = ctile('rinv', [EBLK, NBLK])
            nc.vector.reciprocal(out=rinv[:], in_=rpe[:])
            uv = ctile('uv', [EBLK, 3 * NBLK])
            TT(out=uv[:].rearrange("p (a b) -> p a b", a=3),
               in0=vd[:].rearrange("p (a b) -> p a b", a=3),
               in1=rinv[:].unsqueeze(1).broadcast_to([EBLK, 3, NBLK]), op=OP.mult)

            # bessel: rad[r, blk] = sin((n+1) * pi/c * r) * (sqrt(2/c) * rinv)
            rscl = ctile('rscl', [EBLK, NBLK])
            # negative prefactor absorbs the sign flip from sin(pi*(q-1)) = -sin(pi*q)
            TS(out=rscl[:], in0=rinv[:], scalar1=float(-_RSCL), scalar2=None, op0=OP.mult)
            radp = ctile('radp', [EBLK, N_RBF * NBLK])
            marg = ctile('marg', [EBLK, N_RBF * NBLK])
            TT(out=marg[:].rearrange("p (r b) -> p r b", r=N_RBF),
               in0=rr[:].unsqueeze(1).broadcast_to([EBLK, N_RBF, NBLK]),
               in1=s['nvec'][:].unsqueeze(2).broadcast_to([EBLK, N_RBF, NBLK]),
               op=OP.mult)
            mtmp = ctile('mtmp', [EBLK, N_RBF * NBLK])
            TS(out=mtmp[:], in0=marg[:], scalar1=4.0, scalar2=4.0,
               op0=OP.is_ge, op1=OP.mult)
            TT(out=marg[:], in0=marg[:], in1=mtmp[:], op=OP.subtract)
            TS(out=mtmp[:], in0=marg[:], scalar1=2.0, scalar2=2.0,
               op0=OP.is_ge, op1=OP.mult)
            TT(out=marg[:], in0=marg[:], in1=mtmp[:], op=OP.subtract)
            biaspi = ctile('biaspi', [EBLK, 1])
            nc.vector.memset(biaspi[:], float(-np.pi))
            nc.scalar.activation(out=radp[:], in_=marg[:], func=AF.Sin,
                                 scale=float(np.pi), bias=biaspi[:])
            TT(out=radp[:].rearrange("p (r b) -> p r b", r=N_RBF),
               in0=radp[:].rearrange("p (r b) -> p r b", r=N_RBF),
               in1=rscl[:].unsqueeze(1).broadcast_to([EBLK, N_RBF, NBLK]), op=OP.mult)

            # poly cutoff (p=6); host guarantees u<1
            uu = ctile('uu', [EBLK, NBLK])
            TS(out=uu[:], in0=rr[:], scalar1=float(1.0 / CUTOFF), scalar2=None, op0=OP.mult)
            u3 = ctile('u3', [EBLK, NBLK])
            TT(out=u3[:], in0=uu[:], in1=uu[:], op=OP.mult)
            TT(out=u3[:], in0=u3[:], in1=uu[:], op=OP.mult)
            u6 = ctile('u6', [EBLK, NBLK]); TT(out=u6[:], in0=u3[:], in1=u3[:], op=OP.mult)
            u7 = ctile('u7', [EBLK, NBLK]); TT(out=u7[:], in0=u6[:], in1=uu[:], op=OP.mult)
            u8 = ctile('u8', [EBLK, NBLK]); TT(out=u8[:], in0=u7[:], in1=uu[:], op=OP.mult)
            fc = ctile('fc', [EBLK, NBLK])
            TS(out=fc[:], in0=u6[:], scalar1=-28.0, scalar2=1.0, op0=OP.mult, op1=OP.add)
            t7 = ctile('t7', [EBLK, NBLK])
            TS(out=t7[:], in0=u7[:], scalar1=48.0, scalar2=None, op0=OP.mult)
            TT(out=fc[:], in0=fc[:], in1=t7[:], op=OP.add)
            TS(out=t7[:], in0=u8[:], scalar1=-21.0, scalar2=None, op0=OP.mult)
            TT(out=fc[:], in0=fc[:], in1=t7[:], op=OP.add)

            radf = ctile('radf', [EBLK, N_RBF * NBLK])
            TT(out=radf[:].rearrange("p (r b) -> p r b", r=N_RBF),
               in0=radp[:].rearrange("p (r b) -> p r b", r=N_RBF),
               in1=fc[:].unsqueeze(1).broadcast_to([EBLK, N_RBF, NBLK]), op=OP.mult)

            # onehot [blk, n32]
            onehot = ctile('onehot', [EBLK, NBLK * WIN])
            TT(out=onehot[:].rearrange("p (b n) -> p b n", b=NBLK),
               in0=s['enloc'][:].unsqueeze(2).broadcast_to([EBLK, NBLK, WIN]),
               in1=s['iota32'][:].unsqueeze(1).broadcast_to([EBLK, NBLK, WIN]),
               op=OP.is_equal)

            # enc [blk, ks, kr]
            enc = ctile('enc', [EBLK, NBLK * C])
            TT(out=enc[:].rearrange("p (b i j) -> p b i j", i=K, j=K),
               in0=s['eemb_s'][:].rearrange("p (k b) -> p b k", k=K).unsqueeze(3)
                   .broadcast_to([EBLK, NBLK, K, K]),
               in1=s['eemb_r'][:].rearrange("p (k b) -> p b k", k=K).unsqueeze(2)
                   .broadcast_to([EBLK, NBLK, K, K]),
               op=OP.mult)

            # angular monomials [l, blk]
            ones = ctile('ones', [EBLK, NBLK])
            nc.vector.memset(ones[:], 1.0)
            x2 = ctile('x2', [EBLK, 3 * NBLK])
            TT(out=x2[:], in0=uv[:], in1=uv[:], op=OP.mult)
            x3 = ctile('x3', [EBLK, 3 * NBLK])
            TT(out=x3[:], in0=x2[:], in1=uv[:], op=OP.mult)

            def pow_plane(axis, p_):
                if p_ == 1:
                    return uv[:, axis * NBLK:(axis + 1) * NBLK]
                if p_ == 2:
                    return x2[:, axis * NBLK:(axis + 1) * NBLK]
                return x3[:, axis * NBLK:(axis + 1) * NBLK]

            ang = ctile('ang', [EBLK, NL * NBLK])
            for l in range(NL):
                facs = [pow_plane(a, pw) for a, pw in enumerate((LX[l], LY[l], LZ[l])) if pw > 0]
                dst = ang[:, l * NBLK:(l + 1) * NBLK]
                if len(facs) == 0:
                    nc.scalar.copy(out=dst, in_=ones[:])
                elif len(facs) == 1:
                    nc.scalar.copy(out=dst, in_=facs[0])
                elif len(facs) == 2:
                    TT(out=dst, in0=facs[0], in1=facs[1], op=OP.mult)
                else:
                    TT(out=dst, in0=facs[0], in1=facs[1], op=OP.mult)
                    TT(out=dst, in0=dst, in1=facs[2], op=OP.mult)

            # fp16 casts so the big outer-product TTs run at 2x DVE rate
            ang_h = ctile('ang_h', [EBLK, NL * NBLK], F16)
            nc.scalar.copy(out=ang_h[:], in_=ang[:])
            enc_h = ctile('enc_h', [EBLK, NBLK * C], F16)
            nc.scalar.copy(out=enc_h[:], in_=enc[:])
            radf_h = ctile('radf_h', [EBLK, N_RBF * NBLK], F16)
            nc.scalar.copy(out=radf_h[:], in_=radf[:])
            onehot_h = ctile('onehot_h', [EBLK, NBLK * WIN], F16)
            nc.scalar.copy(out=onehot_h[:], in_=onehot[:])

            # P = ang (x) enc : [blk, l, c] fp16 (chunked so psA0 starts early)
            P = ctile('P', [EBLK, NBLK * LC], F16)
            for b0 in range(0, NBLK, 4):
                TT(out=P[:, b0 * LC:(b0 + 4) * LC]
                       .rearrange("p (b l c) -> p b l c", l=NL, c=C),
                   in0=ang_h[:].rearrange("p (l b) -> p b l", l=NL)
                       [:, b0:b0 + 4].unsqueeze(3)
                       .broadcast_to([EBLK, 4, NL, C]),
                   in1=enc_h[:].rearrange("p (b c) -> p b c", c=C)
                       [:, b0:b0 + 4].unsqueeze(2)
                       .broadcast_to([EBLK, 4, NL, C]),
                   op=OP.mult)

            # lhsT1_g = radf-half (x) onehot : [blk, r4, n32] fp16
            lhsT1 = []
            for g in range(2):
                lt = ctile(f'lhsT1_{g}', [EBLK, NBLK * EBLK], F16)
                lhsT1.append(lt)
            for b0 in range(0, NBLK, 4):
                for g in range(2):
                    lt = lhsT1[g]
                    TT(out=lt[:, b0 * EBLK:(b0 + 4) * EBLK]
                           .rearrange("p (b r n) -> p b r n", r=4, n=WIN),
                       in0=radf_h[:].rearrange("p (r b) -> p b r", r=N_RBF)
                           [:, b0:b0 + 4, g * 4:(g + 1) * 4].unsqueeze(3)
                           .broadcast_to([EBLK, 4, 4, WIN]),
                       in1=onehot_h[:].rearrange("p (b n) -> p b n", b=NBLK)
                           [:, b0:b0 + 4].unsqueeze(2)
                           .broadcast_to([EBLK, 4, 4, WIN]),
                       op=OP.mult)

            # d-major feature tensors: [p, (g2, d-blocks: (w8, l in d, c9))]
            def dmaj_copy(dst, src_ps, w):
                # scatter one window's (g, l, c) psum block into d-major dst
                for d, (ls, lcnt) in enumerate(GRP_SLICES):
                    nc.scalar.copy(
                        out=dst[:].rearrange("p (g q) -> p g q", g=2)
                            [:, :, DOFF[d] + w * lcnt * C:
                                   DOFF[d] + (w + 1) * lcnt * C],
                        in_=src_ps[:].rearrange("p (g q) -> p g q", g=2)
                            [:, :, ls * C:(ls + lcnt) * C])

            # ---- pass 1: per-window segment sum -> A0s2h (w-outer, fp16) ----
            A0s2h = ctile('A0s2h', [EBLK, 2880], F16)
            for w in range(NWINC):
                psA0 = pp.tile([EBLK, GLC], F32, name='psA0', tag='seg')
                for g in range(2):
                    for bi in range(NBW):
                        blk = w * NBW + bi
                        nc.tensor.matmul(
                            out=psA0[:, g * LC:(g + 1) * LC],
                            lhsT=lhsT1[g][:, blk * EBLK:(blk + 1) * EBLK],
                            rhs=P[:, blk * LC:(blk + 1) * LC],
                            start=(bi == 0), stop=(bi == NBW - 1),
                            skip_group_check=True)
                dmaj_copy(A0s2h, psA0, w)

            # ---- window-merged radial-style transforms ----
            def merged_mm(pt_map, terms, emit_start=True, emit_stop=True, gouts=(0, 1)):
                """terms: list of (weight_field, src_tile).  For each psum
                region slice, accumulate  sum_gin  w[gout,d,gin].T @ src[gin]
                for every term.  pt_map: (gout, ti) -> psum tile."""
                for gout in gouts:
                    for ti, (width, slices) in enumerate(RAD_TILES):
                        pt = pt_map[(gout, ti)]
                        for (d, soff, wd, off) in slices:
                            seq = [(wf, st, gin) for wf, st in terms for gin in range(2)]
                            for j, (wf, st, gin) in enumerate(seq):
                                wcol = ((gout * 4 + d) * 2 + gin) * EBLK
                                nc.tensor.matmul(
                                    out=pt[:, off:off + wd],
                                    lhsT=s16[wf][:, wcol:wcol + EBLK],
                                    rhs=st[:, gin * NWINC * LC + soff:
                                           gin * NWINC * LC + soff + wd],
                                    start=(emit_start and j == 0),
                                    stop=(emit_stop and j == len(seq) - 1),
                                    skip_group_check=True)

            A_s2h = ctile('A_s2h', [EBLK, 2880], F16)
            for gout in range(2):
                pts = {(gout, ti): pr.tile([EBLK, wd], F32, name=f'psA_{ti}',
                                           tag=f'rad{ti}')
                       for ti, (wd, _) in enumerate(RAD_TILES)}
                merged_mm(pts, [('wbd_rad', A0s2h)], gouts=(gout,))
                for ti, (wdth, slices) in enumerate(RAD_TILES):
                    for (d, soff, wd, off) in slices:
                        nc.scalar.copy(
                            out=A_s2h[:, gout * 1440 + soff:gout * 1440 + soff + wd],
                            in_=pts[(gout, ti)][:, off:off + wd])

            # ---- symmetrize: Ain d-major -> Bs [p, (g, k5, w, c)] fp32 ----
            def symmetrize(Ain, tagpfx):
                Bs = ctile(tagpfx, [EBLK, 2 * NB * NWINC * C])
                for g in range(2):
                    gb = g * NWINC * LC
                    # nu=1 block: A degree-0 block is already (w, c)
                    nc.scalar.copy(
                        out=Bs[:, (g * NB) * NWINC * C:(g * NB + 1) * NWINC * C],
                        in_=Ain[:, gb:gb + NWINC * C])
                    for d, (ls, lcnt) in enumerate(GRP_SLICES):
                        Av = Ain[:, gb + DOFF[d]:gb + DOFF[d] + NWINC * lcnt * C] \
                            .rearrange("p (w l c) -> p w c l", w=NWINC, l=lcnt)
                        wm2 = ctile(f'{tagpfx}_wm{g}_{d}', [EBLK, NWINC * C * lcnt])
                        wv = wm2[:].rearrange("p (w c l) -> p w c l", w=NWINC, c=C)
                        nc.scalar.activation(out=wv, in_=Av, func=AF.Square)
                        if lcnt > 1:
                            TT(out=wv, in0=wv,
                               in1=s['multi_l'][:, ls:ls + lcnt]
                                   .unsqueeze(1).unsqueeze(2)
                                   .broadcast_to([EBLK, NWINC, C, lcnt]),
                               op=OP.mult)
                        ob = (g * NB + 1 + d) * NWINC * C
                        nc.vector.tensor_reduce(
                            Bs[:, ob:ob + NWINC * C]
                                .rearrange("p (w c) -> p w c", w=NWINC).unsqueeze(3),
                            wv, mybir.AxisListType.X, OP.add)
                return Bs

            B0s = symmetrize(A_s2h, 'B0')
            B0sh = ctile('B0sh', [EBLK, NWINC * 2 * NB * C], F16)
            nc.scalar.copy(out=B0sh[:], in_=B0s[:])

            # ---- chi (all windows at once): psC [32, (w, c)] ----
            psC = pp.tile([WIN, NWINC * C], F32, name='psC', tag='seg')
            first = True
            for g in range(2):
                for k in range(NB):
                    nc.tensor.matmul(
                        out=psC[:],
                        lhsT=s16['wbd_chi'][:, (g * NB + k) * WIN:(g * NB + k + 1) * WIN],
                        rhs=B0sh[:, (g * NB + k) * NWINC * C:
                                 (g * NB + k + 1) * NWINC * C],
                        start=first, stop=(g == 1 and k == NB - 1),
                        skip_group_check=True)
                    first = False
            chiS = ctile('chiS', [WIN, NWINC * C], F16)
            nc.scalar.copy(out=chiS[:], in_=psC[:])
            nc.sync.dma_start(out=outB[:, 0:NWINC * 2 * NB * C], in_=B0s[:])

            # ---- node table -> DRAM, AllGather (fp16, Shared output) ----
            F8 = mybir.dt.float8e4
            T_local = dp.tile([NWINC * WIN, TW], F8, name='T_local')
            T_full = dp.tile([NSLOT, TW], F8, name='T_full', addr_space="Shared")
            # stage [p, (w, g, l, c)] table layout in fp8 (gpsimd, off ACT)
            A_tab = ctile('A_tab', [EBLK, 2880], F8)
            for g in range(2):
                for d, (ls, lcnt) in enumerate(GRP_SLICES):
                    nc.gpsimd.tensor_copy(
                        out=A_tab[:].rearrange("p (w g q) -> p w g q",
                                               w=NWINC, g=2)
                            [:, :, g, ls * C:(ls + lcnt) * C],
                        in_=A_s2h[:, g * 1440 + DOFF[d]:
                                  g * 1440 + DOFF[d] + NWINC * lcnt * C]
                            .rearrange("p (w q) -> p w q", w=NWINC))
            for x in range(4):
                for g in range(2):
                    nc.sync.dma_start(
                        out=T_local[:, 0:RB * LC]
                            .rearrange("(w n) (b q) -> n w b q", n=WIN, b=RB)
                            [:, :, g * 4 + x],
                        in_=A_tab[x * WIN:(x + 1) * WIN, :]
                            .rearrange("p (w g q) -> p w g q", w=NWINC, g=2)
                            [:, :, g])
            nc.sync.dma_start(
                out=T_local[:, RB * LC:RB * LC + 2 * C].bitcast(F16)
                    .rearrange("(w n) c -> n w c", n=WIN),
                in_=chiS[:].rearrange("p (w c) -> p w c", w=NWINC))
            nc.gpsimd.collective_compute(
                "AllGather", mybir.AluOpType.bypass,
                replica_groups=[list(range(N_CORES))],
                ins=[T_local[:].opt()], outs=[T_full[:].opt()])

            # fr = (radf @ W_ar) * MP_NORM : [blk, b8] fp32
            frA = ctile('frA', [EBLK, NBLK * N_RBF])
            frB = ctile('frB', [EBLK, NBLK * N_RBF])
            frt = ctile('frt', [EBLK, NBLK * N_RBF])
            for r_ in range(N_RBF):
                radv = radf[:, r_ * NBLK:(r_ + 1) * NBLK].unsqueeze(2) \
                    .broadcast_to([EBLK, NBLK, N_RBF])
                warv = s['war_mp'][:, r_ * N_RBF:(r_ + 1) * N_RBF].unsqueeze(1) \
                    .broadcast_to([EBLK, NBLK, N_RBF])
                if r_ == 0:
                    TT(out=frA[:].rearrange("p (b k) -> p b k", k=N_RBF),
                       in0=radv, in1=warv, op=OP.mult)
                else:
                    TT(out=frt[:].rearrange("p (b k) -> p b k", k=N_RBF),
                       in0=radv, in1=warv, op=OP.mult)
                    src, dst = (frA, frB) if r_ % 2 == 1 else (frB, frA)
                    TT(out=dst[:], in0=src[:], in1=frt[:], op=OP.add)
            fr = frB   # 7 ping-pong adds end in frB
            fr_h = ctile('fr_h', [EBLK, NBLK * N_RBF], F16)
            nc.scalar.copy(out=fr_h[:], in_=fr[:])

            # lhsT_ar_g = fr-half (x) onehot : [blk, x4, n32] fp16
            lhsT_ar = []
            for g in range(2):
                lt = ctile(f'lhsT_ar_{g}', [EBLK, NBLK * EBLK],
                           mybir.dt.float8e4)
                TT(out=lt[:].rearrange("p (w b n) -> p w b n", b=4, n=WIN),
                   in0=fr_h[:].rearrange("p (w k) -> p w k", k=N_RBF)
                       [:, :, g * 4:(g + 1) * 4].unsqueeze(3)
                       .broadcast_to([EBLK, NBLK, 4, WIN]),
                   in1=onehot_h[:].rearrange("p (w n) -> p w n", w=NBLK).unsqueeze(2)
                       .broadcast_to([EBLK, NBLK, 4, WIN]),
                   op=OP.mult)
                lhsT_ar.append(lt)



            # ---- psMem for both gouts overlaps the AllGather ----
            Mem_s2 = ctile('Mem_s2', [EBLK, 2880])
            for gout in range(2):
                pmm = {(gout, ti): pr.tile([EBLK, wd], F32, name=f'psMm{gout}_{ti}',
                                           tag=f'rad{ti}')
                       for ti, (wd, _) in enumerate(RAD_TILES)}
                merged_mm(pmm, [('wbd_mem', A_s2h)], gouts=(gout,))
                for ti, (wdth, slices) in enumerate(RAD_TILES):
                    for (d, soff, wd, off) in slices:
                        nc.scalar.copy(
                            out=Mem_s2[:, gout * 1440 + soff:gout * 1440 + soff + wd],
                            in_=pmm[(gout, ti)][:, off:off + wd])

            # ---- pass 2 per-window: gather -> P2 -> psB0 -> psAr(Q) ----
            Ab0s2h = ctile('Ab0s2h', [EBLK, 2880], F16)
            Ar_s2 = ctile('Ar_s2', [EBLK, 2880])
            for w in range(NWINC):
                ags = []
                P2s = []
                for bi in range(NBW):
                    blk = w * NBW + bi
                    ag8 = gp.tile([EBLK, TW], F8, name='ag8', tag='ag8')
                    nc.gpsimd.indirect_dma_start(
                        out=ag8[:], out_offset=None, in_=T_full[:],
                        in_offset=bass.IndirectOffsetOnAxis(
                            ap=esrow_s[:, blk:blk + 1], axis=0))
                    if debug and w == 0 and bi == 0:
                        nc.sync.dma_start(out=dbg['ag0'][:], in_=ag8[:])
                    ags.append(ag8)
                    P2 = wp.tile([EBLK, LC], F16, name='P2', tag='P2')
                    TT(out=P2[:].rearrange("p (l c) -> p l c", c=C),
                       in0=P[:, blk * LC:(blk + 1) * LC].rearrange("p (l c) -> p l c", c=C),
                       in1=ag8[:, RB * LC:RB * LC + 2 * C].bitcast(F16)
                           .unsqueeze(1).broadcast_to([EBLK, NL, C]),
                       op=OP.mult)
                    P2s.append(P2)
                psB0 = pp.tile([EBLK, GLC], F32, name='psB0', tag='seg')
                for g in range(2):
                    for bi in range(NBW):
                        blk = w * NBW + bi
                        nc.tensor.matmul(
                            out=psB0[:, g * LC:(g + 1) * LC],
                            lhsT=lhsT1[g][:, blk * EBLK:(blk + 1) * EBLK],
                            rhs=P2s[bi][:],
                            start=(bi == 0), stop=(bi == NBW - 1),
                            skip_group_check=True)
                dmaj_copy(Ab0s2h, psB0, w)
                psAr = pp.tile([EBLK, GLC], F32, name='psAr', tag='ar')
                for g in range(2):
                    for x in range(4):
                        for bi in range(NBW):
                            blk = w * NBW + bi
                            nc.tensor.matmul(
                                out=psAr[x * WIN:(x + 1) * WIN, g * LC:(g + 1) * LC],
                                lhsT=lhsT_ar[g][:, blk * EBLK + x * WIN:
                                                blk * EBLK + (x + 1) * WIN],
                                rhs=ags[bi][:, (g * 4 + x) * LC:(g * 4 + x + 1) * LC],
                                start=(bi == 0), stop=(bi == NBW - 1),
                                skip_group_check=True,
                                tile_position=(0, x * WIN))
                dmaj_copy(Ar_s2, psAr, w)

            # ---- merged psAb, then combine with Ar and Mem ----
            Anew = ctile('Anew', [EBLK, 2880])
            for gout in range(2):
                pm = {(gout, ti): pr.tile([EBLK, wd], F32, name=f'psM{gout}_{ti}',
                                          tag=f'rad{ti}')
                      for ti, (wd, _) in enumerate(RAD_TILES)}
                merged_mm(pm, [('wbd_radmp', Ab0s2h)], gouts=(gout,))
                for ti, (wdth, slices) in enumerate(RAD_TILES):
                    for (d, soff, wd, off) in slices:
                        gs = slice(gout * 1440 + soff, gout * 1440 + soff + wd)
                        TT(out=Anew[:, gs],
                           in0=pm[(gout, ti)][:, off:off + wd],
                           in1=Ar_s2[:, gs], op=OP.add)
                        nc.gpsimd.tensor_tensor(
                            out=Anew[:, gs], in0=Anew[:, gs],
                            in1=Mem_s2[:, gs], op=OP.add)

            B1s = symmetrize(Anew, 'B1')

            if debug:
                nc.sync.dma_start(out=dbg['A_s'][:], in_=A_s2h[:])
                nc.sync.dma_start(out=dbg['A0s'][:], in_=A0s2h[:])
                nc.sync.dma_start(out=dbg['Anew'][:], in_=Anew[:])
                nc.sync.dma_start(out=dbg['chiS'][:], in_=chiS[:])
                nc.sync.dma_start(out=dbg['B0s'][:], in_=B0s[:])
                nc.sync.dma_start(out=dbg['P'][:], in_=P[:])
                nc.sync.dma_start(out=dbg['Ab0'][:], in_=Ab0s2h[:])
                nc.sync.dma_start(out=dbg['Ar'][:], in_=Ar_s2[:])
                nc.sync.dma_start(out=dbg['radf'][:], in_=radf[:])
                nc.sync.dma_start(out=dbg['fr'][:], in_=fr[:])
                nc.sync.dma_start(out=dbg['Tloc'][:], in_=T_local[:])

            # ---- output (B0 half already written right after pass 1) ----
            half = NWINC * 2 * NB * C
            nc.sync.dma_start(out=outB[:, half:2 * half], in_=B1s[:])

    nc.compile()
    return nc


_CACHE = {}


def kernel(**inputs) -> np.ndarray:
    return _kernel_impl(inputs, debug=False)[0]


def _kernel_impl(inputs, debug=False, trace=False):
    from concourse.bass_utils import run_bass_kernel_spmd

    packed, slot_of_node = _host_prep(inputs)

    key = 'ncdbg' if debug else 'nc'
    if key not in _CACHE:
        _CACHE[key] = _build_program(debug=debug)
    nc = _CACHE[key]

    in_maps = [dict(p) for p in packed]

    res = run_bass_kernel_spmd(nc, in_maps, core_ids=list(range(N_CORES)),
                               trace=trace)

    feats_slots = np.zeros((NSLOT, RB, NB, C, 2), np.float32)
    for ci in range(N_CORES):
        # outB [128, (t2, g2, k5, w8, c9)]; partition p = x*32 + n
        arr = res.results[ci]['outB'].reshape(4, WIN, 2, 2, NB, NWINC, C)
        # [x, n, t, g, k, w, c] -> [w, n, (g,x)=b, k, c, t]
        arr = np.transpose(arr, (5, 1, 3, 0, 4, 6, 2))
        feats_slots[ci * NWINC * WIN:(ci + 1) * NWINC * WIN] = \
            arr.reshape(NWINC * WIN, RB, NB, C, 2)
    return feats_slots[slot_of_node], res


if __name__ == '__main__':
    import pickle, os
    if os.path.exists('/tmp/inputs.pkl'):
        inputs = pickle.load(open('/tmp/inputs.pkl', 'rb'))
    else:
        import reference as Rf
        inputs = {k: np.asarray(v) for k, v in Rf.setup_inputs().items()}
        pickle.dump(inputs, open('/tmp/inputs.pkl', 'wb'))
    out = kernel(**inputs)
    print("kernel out", out.shape, out.dtype, float(np.abs(out).max()))
    if os.path.exists('/tmp/expected.npy'):
        exp = np.load('/tmp/expected.npy')
        err = np.abs(out - exp).max()
        print("max abs err vs expected:", err, "rel:", err / np.abs(exp).max())


# revision 10
# speedup vs baseline: 1.2140x; 1.2140x over previous
"""Trainium2 Bass kernel for nn_Cace_74569222193773 (CACE GNN message passing).

8-core SPMD, recv-partitioned graph (atoms in 64 edge-balanced windows of 32,
8 windows/core; edges live with their receiver window, 2x128-edge blocks per
window), AllGather halo of sender-node features, replicated weights.

Performance design (vs fp32 per-window baseline, ~2.4x faster):
  * All PE matmuls in fp16 (1 cyc/col vs fp32's 2 half-speed passes), fp32
    PSUM accumulation; A_ar path in fp8e4.  numpy-simulated end-to-end
    rel err ~6e-3 vs the 2e-2 gate.
  * Node features in a d-major layout [p=(x4,n32), (g2, per-degree blocks of
    (w8, l-in-d, c9))] so the radial / memory / chi transforms run as
    window-merged wide matmuls with flat contiguous rhs (20 MMs instead of
    128 per transform) and all psum->SBUF copies are contiguous.
  * AllGather table in fp8e4 (1440B A row + 18B fp16 chi via bitcast),
    Shared-space output: ~3MB on the wire instead of 12MB fp32.
  * psMem matmuls and the B0-half output DMA overlap the AllGather window.

kernel() takes FULL unsharded inputs, returns FULL [2000,8,5,9,2] fp32.
"""
import heapq
from math import factorial

import numpy as np

# ---- static problem config (mirrors the reference) ----
MAX_L = 3; N_RBF = 8; RB = 8; K = 3
CUTOFF = 5.5
N_NODES = 2000
MP_NORM = 1.0 / np.sqrt(25.0)
C = K * K                      # 9
NB = 1 + (MAX_L + 1)           # 5

def _lxlylz(max_l):
    out = []
    for l in range(max_l + 1):
        for lx in range(l, -1, -1):
            for ly in range(l - lx, -1, -1):
                out.append((lx, ly, l - lx - ly))
    return out

L_LIST = _lxlylz(MAX_L); NL = len(L_LIST)                       # 20
LX = np.array([t[0] for t in L_LIST]); LY = np.array([t[1] for t in L_LIST])
LZ = np.array([t[2] for t in L_LIST]); DEGS = LX + LY + LZ
MULTI = np.array([factorial(int(d)) / (factorial(int(a)) * factorial(int(b)) * factorial(int(c)))
                  for a, b, c, d in zip(LX, LY, LZ, DEGS)], dtype=np.float32)
GRP_SLICES = []                 # (l_start, l_count) per degree; DEGS is sorted
for d in range(MAX_L + 1):
    idx = np.where(DEGS == d)[0]
    GRP_SLICES.append((int(idx[0]), int(len(idx))))

# ---- sharding geometry ----
N_CORES = 8
WIN = 32                        # nodes per window
NWINC = 8                       # windows per core
NWIN = N_CORES * NWINC          # 64
NSLOT = NWIN * WIN              # 2048 node slots
EBLK = 128                      # edges per block (partition dim)
NBW = 2                         # blocks per window
NBLK = NWINC * NBW              # 16 blocks per core
EPAD = NBLK * EBLK              # 2048 edge slots per core
LC = NL * C                     # 180
GLC = 2 * LC                    # 360 = per-window feature block (g, l, c)
TW = RB * LC + 32               # fp8 table row: 1440 A bytes + 18B fp16 chi + pad

# d-major feature layout: g-block (1440) = concat over degree d of a block
# ordered (w8, l in d, c9).  DOFF[d] = elem offset of block d inside a g-block.
DOFF = [0, 72, 288, 720]
# psum tiles for the window-merged radial transforms: flat contiguous slices
# (tile_width, [(d, src_off_in_gblock, width, psum_off), ...]); psum offsets
# keep each matmul slice inside one 2KB bank.
RAD_TILES = [
    (288,  [(0, 0, 72, 0), (1, 72, 216, 72)]),
    (432,  [(2, 288, 432, 0)]),
    (1024, [(3, 720, 360, 0), (3, 1080, 360, 512)]),
]

_PCUT = np.pi / CUTOFF
_RSCL = np.sqrt(2.0 / CUTOFF)

# packed-input column layout: per-core fp32 fields then replicated consts
F32_FIELDS = ['exyz_s', 'exyz_r', 'eemb_s', 'eemb_r', 'enloc']
CONST_FIELDS = ['iota32', 'multi_l', 'war_mp', 'nvec']
F16_FIELDS = ['wbd_rad', 'wbd_radmp', 'wbd_mem', 'wbd_chi']
FIELD_W = dict(exyz_s=3 * NBLK, exyz_r=3 * NBLK, eemb_s=3 * NBLK, eemb_r=3 * NBLK,
               enloc=NBLK, iota32=WIN, multi_l=NL, war_mp=64, nvec=N_RBF,
               wbd_rad=2048, wbd_radmp=2048, wbd_mem=2048, wbd_chi=2 * NB * WIN)
FIELD_OFF = {}
_o = 0
for _f in F32_FIELDS + CONST_FIELDS:
    FIELD_OFF[_f] = _o
    _o += FIELD_W[_f]
TOTW = _o
_o = 0
for _f in F16_FIELDS:
    FIELD_OFF[_f] = _o
    _o += FIELD_W[_f]
TOTW16 = _o


def _host_prep(inputs):
    pos = np.asarray(inputs['positions'], np.float32)
    shifts = np.asarray(inputs['shifts'], np.float32)
    W_embed = np.asarray(inputs['W_embed'], np.float32)
    species = np.asarray(inputs['species'])
    ei = np.asarray(inputs['edge_index'])
    send, recv = ei[0], ei[1]

    vec = (pos[recv] + shifts - pos[send]).astype(np.float64)
    r = np.sqrt((vec * vec).sum(-1))
    keep = np.where(r < CUTOFF)[0]
    deg = np.bincount(recv[keep], minlength=N_NODES)

    # balanced node->window assignment (<=WIN nodes, balance edge load)
    order = np.argsort(-deg, kind='stable')
    win_cnt = np.zeros(NWIN, np.int64); win_load = np.zeros(NWIN, np.int64)
    win_of_node = np.zeros(N_NODES, np.int64); pos_in_win = np.zeros(N_NODES, np.int64)
    heap = [(0, w) for w in range(NWIN)]
    heapq.heapify(heap)
    for nd in order:
        popped = []
        while True:
            load, w = heapq.heappop(heap)
            if win_cnt[w] < WIN:
                break
            popped.append((load, w))
        for it in popped:
            heapq.heappush(heap, it)
        win_of_node[nd] = w; pos_in_win[nd] = win_cnt[w]
        win_cnt[w] += 1; win_load[w] += deg[nd]
        heapq.heappush(heap, (win_load[w], w))
    if win_load.max() > NBW * EBLK:
        raise RuntimeError(f"window overflow: {win_load.max()} > {NBW * EBLK}")

    slot_of_node = win_of_node * WIN + pos_in_win
    emb = W_embed[species]                       # [N, K]

    win_edges = [[] for _ in range(NWIN)]
    rk = recv[keep]
    for i, e in enumerate(keep):
        win_edges[win_of_node[rk[i]]].append(e)

    cores = []
    for ci in range(N_CORES):
        e_xyz_s = np.zeros((EPAD, 3), np.float32)
        e_xyz_r = np.zeros((EPAD, 3), np.float32)
        e_emb_s = np.zeros((EPAD, K), np.float32)
        e_emb_r = np.zeros((EPAD, K), np.float32)
        e_nloc = np.full((EPAD,), -1.0, np.float32)
        e_srow = np.zeros((EPAD,), np.int32)
        e_xyz_r[:, 0] = 1.0                      # pads: r = 1, finite math
        for wl in range(NWINC):
            w = ci * NWINC + wl
            eids = np.array(win_edges[w], dtype=np.int64)
            base = wl * NBW * EBLK
            cnt = len(eids)
            if cnt:
                sl = slice(base, base + cnt)
                e_xyz_s[sl] = pos[send[eids]]
                e_xyz_r[sl] = pos[recv[eids]] + shifts[eids]
                e_emb_s[sl] = emb[send[eids]]
                e_emb_r[sl] = emb[recv[eids]]
                e_nloc[sl] = pos_in_win[recv[eids]].astype(np.float32)
                e_srow[sl] = slot_of_node[send[eids]].astype(np.int32)

        # device layout: edge e = blk*128 + p  ->  [128, NBLK(, d)]
        def dev(x):
            if x.ndim == 1:
                return np.ascontiguousarray(x.reshape(NBLK, EBLK).T)
            return np.ascontiguousarray(np.transpose(x.reshape(NBLK, EBLK, -1), (1, 0, 2)))

        # axis-major planes [128, 3*NBLK] = [a, blk]
        def axmajor(x3):
            d = dev(x3)                                  # [128, NBLK, 3]
            return np.ascontiguousarray(np.transpose(d, (0, 2, 1)).reshape(EBLK, 3 * NBLK))

        cores.append(dict(
            exyz_s=axmajor(e_xyz_s), exyz_r=axmajor(e_xyz_r),
            eemb_s=axmajor(e_emb_s), eemb_r=axmajor(e_emb_r),
            enloc=np.ascontiguousarray(dev(e_nloc)),
            esrow=np.ascontiguousarray(dev(e_srow)),
            raw=dict(e_xyz_s=e_xyz_s.copy(), e_xyz_r=e_xyz_r.copy(),
                     e_emb_s=e_emb_s.copy(), e_emb_r=e_emb_r.copy(),
                     e_nloc=e_nloc.copy(), e_srow=e_srow.copy()),
        ))

    Wr = np.asarray(inputs['W_radial'], np.float32)   # [4(deg), 8(r), 8(b)]
    Wm = np.asarray(inputs['W_mem'], np.float32)
    Wc = np.asarray(inputs['W_chi'], np.float32)      # [8(b), 5(k)]
    Wa = np.asarray(inputs['W_ar'], np.float32)       # [8(r), 8(b)]
    I32 = np.eye(WIN, dtype=np.float32)

    def bd(W):
        # [4,8,8] -> [128, (gout,d,gin)*128]: kron(W[d, gin*4:+4, gout*4:+4], I32)
        cols = []
        for gout in range(2):
            for d in range(4):
                for gin in range(2):
                    cols.append(np.kron(W[d, gin * 4:gin * 4 + 4, gout * 4:gout * 4 + 4], I32))
        return np.concatenate(cols, axis=1)          # [128, 2048]

    wchi_cols = []
    for g in range(2):
        for k in range(NB):
            wchi_cols.append(np.kron(Wc[g * 4:g * 4 + 4, k:k + 1], I32))   # [128, 32]
    consts32 = dict(
        multi_l=np.tile(MULTI.reshape(1, NL), (EBLK, 1)),            # [128, 20]
        iota32=np.tile(np.arange(WIN, dtype=np.float32).reshape(1, WIN), (EBLK, 1)),
        war_mp=np.tile((Wa * MP_NORM).reshape(1, 64), (EBLK, 1)),    # [128, 64] (r-major)
        nvec=np.tile((np.arange(1, N_RBF + 1, dtype=np.float32) / CUTOFF).reshape(1, N_RBF),
                     (EBLK, 1)),
    )
    consts16 = dict(
        wbd_rad=bd(Wr),
        wbd_radmp=bd(Wr * MP_NORM),
        wbd_mem=bd(Wm),
        wbd_chi=np.concatenate(wchi_cols, axis=1),                   # [128, 320]
    )
    edata16 = np.ascontiguousarray(
        np.concatenate([consts16[nm] for nm in F16_FIELDS], axis=1), np.float16)
    packed = []
    for ci in range(N_CORES):
        cols = [cores[ci][nm] for nm in F32_FIELDS]
        cols += [consts32[nm] for nm in CONST_FIELDS]
        packed.append(dict(edata=np.ascontiguousarray(np.concatenate(cols, axis=1), np.float32),
                           edata16=edata16,
                           esrow=cores[ci]['esrow']))
    _host_prep.aux = dict(cores=cores, slot_of_node=slot_of_node)
    return packed, slot_of_node


def _build_program(debug=False):
    import concourse.bass as bass
    import concourse.mybir as mybir
    from concourse import bacc
    from concourse.tile import TileContext

    F32 = mybir.dt.float32
    F16 = mybir.dt.float16
    AF = mybir.ActivationFunctionType
    OP = mybir.AluOpType

    nc = bacc.Bacc("TRN2", target_bir_lowering=False, debug=False,
                   num_devices=N_CORES)

    edata = nc.dram_tensor('edata', [EBLK, TOTW], F32, kind="ExternalInput")
    edata16_d = nc.dram_tensor('edata16', [EBLK, TOTW16], F16, kind="ExternalInput")
    esrow_d = nc.dram_tensor('esrow', [EBLK, NBLK], mybir.dt.int32, kind="ExternalInput")
    if debug:
        dbg = {nm: nc.dram_tensor('dbg_' + nm, sh, dt, kind="ExternalOutput")
               for nm, sh, dt in [
                   ('A_s', [EBLK, 2880], F16),
                   ('A0s', [EBLK, 2880], F16),
                   ('Anew', [EBLK, 2880], F32),
                   ('chiS', [WIN, NWINC * C], F16),
                   ('B0s', [EBLK, 2 * NB * NWINC * C], F32),
                   ('ag0', [EBLK, TW], mybir.dt.float8e4),
                   ('P', [EBLK, NBLK * LC], F16),
                   ('Ab0', [EBLK, 2880], F16),
                   ('Ar', [EBLK, 2880], F32),
                   ('radf', [EBLK, N_RBF * NBLK], F32),
                   ('fr', [EBLK, NBLK * N_RBF], F32),
                   ('Tloc', [NWINC * WIN, TW], mybir.dt.float8e4)]}
    outB = nc.dram_tensor('outB', [EBLK, 2 * 2 * NB * NWINC * C], F32,
                          kind="ExternalOutput")

    with TileContext(nc) as tc:
        with (tc.tile_pool(name="const", bufs=1) as cp,
              tc.tile_pool(name="work", bufs=4) as wp,
              tc.tile_pool(name="gat", bufs=4) as gp,
              tc.tile_pool(name="psum", bufs=2, space="PSUM") as pp,
              tc.tile_pool(name="psrad", bufs=1, space="PSUM") as pr,
              tc.tile_pool(name="dram", bufs=1, space="DRAM") as dp):

            big = cp.tile([EBLK, TOTW], F32, name='big', tag='big')
            nc.sync.dma_start(out=big[:], in_=edata[:])
            big16 = cp.tile([EBLK, TOTW16], F16, name='big16', tag='big16')
            nc.sync.dma_start(out=big16[:], in_=edata16_d[:])
            esrow_s = cp.tile([EBLK, NBLK], mybir.dt.int32, name='esrow_s', tag='esrow_s')
            nc.sync.dma_start(out=esrow_s[:], in_=esrow_d[:])

            # tiny warm-up AllGather: absorbs first-collective setup latency
            # on the idle CC queue while edge prep runs
            warm_l = dp.tile([8, 16], F32, name='warm_l')
            warm_f = dp.tile([64, 16], F32, name='warm_f', addr_space="Shared")
            nc.sync.dma_start(out=warm_l[:], in_=big[0:8, 0:16])
            nc.gpsimd.collective_compute(
                "AllGather", mybir.AluOpType.bypass,
                replica_groups=[list(range(N_CORES))],
                ins=[warm_l[:].opt()], outs=[warm_f[:].opt()])

            class _S:
                def __init__(self, t):
                    self.t = t
                def __getitem__(self, nm):
                    off = FIELD_OFF[nm]
                    return self.t[:, off:off + FIELD_W[nm]]
            s = _S(big)
            s16 = _S(big16)

            def ctile(tag, shape, dtype=F32):
                return cp.tile(shape, dtype, name=tag, tag=tag)

            TT = nc.vector.tensor_tensor
            TS = nc.vector.tensor_scalar

            # ---- geometry, edge-major [128, a*NBLK+blk] ----
            vd = ctile('vd', [EBLK, 3 * NBLK])
            TT(out=vd[:], in0=s['exyz_r'][:], in1=s['exyz_s'][:], op=OP.subtract)
            sq = ctile('sq', [EBLK, 3 * NBLK])
            TT(out=sq[:], in0=vd[:], in1=vd[:], op=OP.mult)
            r2 = ctile('r2', [EBLK, NBLK])
            TT(out=r2[:], in0=sq[:, 0:NBLK], in1=sq[:, NBLK:2 * NBLK], op=OP.add)
            TT(out=r2[:], in0=r2[:], in1=sq[:, 2 * NBLK:3 * NBLK], op=OP.add)
            rr = ctile('rr', [EBLK, NBLK])
            nc.scalar.activation(out=rr[:], in_=r2[:], func=AF.Sqrt)
            rpe = ctile('rpe', [EBLK, NBLK])
            TS(out=rpe[:], in0=rr[:], scalar1=1e-9, scalar2=None, op0=OP.add)
            rinv = ctile('rinv', [EBLK, NBLK])
            nc.vector.reciprocal(out=rinv[:], in_=rpe[:])
            uv = ctile('uv', [EBLK, 3 * NBLK])
            TT(out=uv[:].rearrange("p (a b) -> p a b", a=3),
               in0=vd[:].rearrange("p (a b) -> p a b", a=3),
               in1=rinv[:].unsqueeze(1).broadcast_to([EBLK, 3, NBLK]), op=OP.mult)

            # bessel: rad[r, blk] = sin((n+1) * pi/c * r) * (sqrt(2/c) * rinv)
            rscl = ctile('rscl', [EBLK, NBLK])
            # negative prefactor absorbs the sign flip from sin(pi*(q-1)) = -sin(pi*q)
            TS(out=rscl[:], in0=rinv[:], scalar1=float(-_RSCL), scalar2=None, op0=OP.mult)
            radp = ctile('radp', [EBLK, N_RBF * NBLK])
            marg = ctile('marg', [EBLK, N_RBF * NBLK])
            TT(out=marg[:].rearrange("p (r b) -> p r b", r=N_RBF),
               in0=rr[:].unsqueeze(1).broadcast_to([EBLK, N_RBF, NBLK]),
               in1=s['nvec'][:].unsqueeze(2).broadcast_to([EBLK, N_RBF, NBLK]),
               op=OP.mult)
            mtmp = ctile('mtmp', [EBLK, N_RBF * NBLK])
            TS(out=mtmp[:], in0=marg[:], scalar1=4.0, scalar2=4.0,
               op0=OP.is_ge, op1=OP.mult)
            TT(out=marg[:], in0=marg[:], in1=mtmp[:], op=OP.subtract)
            TS(out=mtmp[:], in0=marg[:], scalar1=2.0, scalar2=2.0,
               op0=OP.is_ge, op1=OP.mult)
            TT(out=marg[:], in0=marg[:], in1=mtmp[:], op=OP.subtract)
            biaspi = ctile('biaspi', [EBLK, 1])
            nc.vector.memset(biaspi[:], float(-np.pi))
            nc.scalar.activation(out=radp[:], in_=marg[:], func=AF.Sin,
                                 scale=float(np.pi), bias=biaspi[:])
            TT(out=radp[:].rearrange("p (r b) -> p r b", r=N_RBF),
               in0=radp[:].rearrange("p (r b) -> p r b", r=N_RBF),
               in1=rscl[:].unsqueeze(1).broadcast_to([EBLK, N_RBF, NBLK]), op=OP.mult)

            # poly cutoff (p=6); host guarantees u<1
            uu = ctile('uu', [EBLK, NBLK])
            TS(out=uu[:], in0=rr[:], scalar1=float(1.0 / CUTOFF), scalar2=None, op0=OP.mult)
            u3 = ctile('u3', [EBLK, NBLK])
            TT(out=u3[:], in0=uu[:], in1=uu[:], op=OP.mult)
            TT(out=u3[:], in0=u3[:], in1=uu[:], op=OP.mult)
            u6 = ctile('u6', [EBLK, NBLK]); TT(out=u6[:], in0=u3[:], in1=u3[:], op=OP.mult)
            u7 = ctile('u7', [EBLK, NBLK]); TT(out=u7[:], in0=u6[:], in1=uu[:], op=OP.mult)
            u8 = ctile('u8', [EBLK, NBLK]); TT(out=u8[:], in0=u7[:], in1=uu[:], op=OP.mult)
            fc = ctile('fc', [EBLK, NBLK])
            TS(out=fc[:], in0=u6[:], scalar1=-28.0, scalar2=1.0, op0=OP.mult, op1=OP.add)
            t7 = ctile('t7', [EBLK, NBLK])
            TS(out=t7[:], in0=u7[:], scalar1=48.0, scalar2=None, op0=OP.mult)
            TT(out=fc[:], in0=fc[:], in1=t7[:], op=OP.add)
            TS(out=t7[:], in0=u8[:], scalar1=-21.0, scalar2=None, op0=OP.mult)
            TT(out=fc[:], in0=fc[:], in1=t7[:], op=OP.add)

            radf = ctile('radf', [EBLK, N_RBF * NBLK])
            TT(out=radf[:].rearrange("p (r b) -> p r b", r=N_RBF),
               in0=radp[:].rearrange("p (r b) -> p r b", r=N_RBF),
               in1=fc[:].unsqueeze(1).broadcast_to([EBLK, N_RBF, NBLK]), op=OP.mult)

            # onehot [blk, n32]
            onehot = ctile('onehot', [EBLK, NBLK * WIN])
            TT(out=onehot[:].rearrange("p (b n) -> p b n", b=NBLK),
               in0=s['enloc'][:].unsqueeze(2).broadcast_to([EBLK, NBLK, WIN]),
               in1=s['iota32'][:].unsqueeze(1).broadcast_to([EBLK, NBLK, WIN]),
               op=OP.is_equal)

            # enc [blk, ks, kr]
            enc = ctile('enc', [EBLK, NBLK * C])
            TT(out=enc[:].rearrange("p (b i j) -> p b i j", i=K, j=K),
               in0=s['eemb_s'][:].rearrange("p (k b) -> p b k", k=K).unsqueeze(3)
                   .broadcast_to([EBLK, NBLK, K, K]),
               in1=s['eemb_r'][:].rearrange("p (k b) -> p b k", k=K).unsqueeze(2)
                   .broadcast_to([EBLK, NBLK, K, K]),
               op=OP.mult)

            # angular monomials [l, blk]
            ones = ctile('ones', [EBLK, NBLK])
            nc.vector.memset(ones[:], 1.0)
            x2 = ctile('x2', [EBLK, 3 * NBLK])
            TT(out=x2[:], in0=uv[:], in1=uv[:], op=OP.mult)
            x3 = ctile('x3', [EBLK, 3 * NBLK])
            TT(out=x3[:], in0=x2[:], in1=uv[:], op=OP.mult)

            def pow_plane(axis, p_):
                if p_ == 1:
                    return uv[:, axis * NBLK:(axis + 1) * NBLK]
                if p_ == 2:
                    return x2[:, axis * NBLK:(axis + 1) * NBLK]
                return x3[:, axis * NBLK:(axis + 1) * NBLK]

            ang = ctile('ang', [EBLK, NL * NBLK])
            for l in range(NL):
                facs = [pow_plane(a, pw) for a, pw in enumerate((LX[l], LY[l], LZ[l])) if pw > 0]
                dst = ang[:, l * NBLK:(l + 1) * NBLK]
                if len(facs) == 0:
                    nc.scalar.copy(out=dst, in_=ones[:])
                elif len(facs) == 1:
                    nc.scalar.copy(out=dst, in_=facs[0])
                elif len(facs) == 2:
                    TT(out=dst, in0=facs[0], in1=facs[1], op=OP.mult)
                else:
                    TT(out=dst, in0=facs[0], in1=facs[1], op=OP.mult)
                    TT(out=dst, in0=dst, in1=facs[2], op=OP.mult)

            # fp16 casts so the big outer-product TTs run at 2x DVE rate
            ang_h = ctile('ang_h', [EBLK, NL * NBLK], F16)
            nc.scalar.copy(out=ang_h[:], in_=ang[:])
            enc_h = ctile('enc_h', [EBLK, NBLK * C], F16)
            nc.scalar.copy(out=enc_h[:], in_=enc[:])
            radf_h = ctile('radf_h', [EBLK, N_RBF * NBLK], F16)
            nc.scalar.copy(out=radf_h[:], in_=radf[:])
            onehot_h = ctile('onehot_h', [EBLK, NBLK * WIN], F16)
            nc.scalar.copy(out=onehot_h[:], in_=onehot[:])

            # P = ang (x) enc : [blk, l, c] fp16 (chunked so psA0 starts early)
            P = ctile('P', [EBLK, NBLK * LC], F16)
            for b0 in range(0, NBLK, 4):
                TT(out=P[:, b0 * LC:(b0 + 4) * LC]
                       .rearrange("p (b l c) -> p b l c", l=NL, c=C),
                   in0=ang_h[:].rearrange("p (l b) -> p b l", l=NL)
                       [:, b0:b0 + 4].unsqueeze(3)
                       .broadcast_to([EBLK, 4, NL, C]),
                   in1=enc_h[:].rearrange("p (b c) -> p b c", c=C)
                       [:, b0:b0 + 4].unsqueeze(2)
                       .broadcast_to([EBLK, 4, NL, C]),
                   op=OP.mult)

            # lhsT1_g = radf-half (x) onehot : [blk, r4, n32] fp16
            lhsT1 = []
            for g in range(2):
                lt = ctile(f'lhsT1_{g}', [EBLK, NBLK * EBLK], F16)
                lhsT1.append(lt)
            for b0 in range(0, NBLK, 4):
                for g in range(2):
                    lt = lhsT1[g]
                    TT(out=lt[:, b0 * EBLK:(b0 + 4) * EBLK]
                           .rearrange("p (b r n) -> p b r n", r=4, n=WIN),
                       in0=radf_h[:].rearrange("p (r b) -> p b r", r=N_RBF)
                           [:, b0:b0 + 4, g * 4:(g + 1) * 4].unsqueeze(3)
                           .broadcast_to([EBLK, 4, 4, WIN]),
                       in1=onehot_h[:].rearrange("p (b n) -> p b n", b=NBLK)
                           [:, b0:b0 + 4].unsqueeze(2)
                           .broadcast_to([EBLK, 4, 4, WIN]),
                       op=OP.mult)

            # d-major feature tensors: [p, (g2, d-blocks: (w8, l in d, c9))]
            def dmaj_copy(dst, src_ps, w):
                # scatter one window's (g, l, c) psum block into d-major dst
                for d, (ls, lcnt) in enumerate(GRP_SLICES):
                    nc.scalar.copy(
                        out=dst[:].rearrange("p (g q) -> p g q", g=2)
                            [:, :, DOFF[d] + w * lcnt * C:
                                   DOFF[d] + (w + 1) * lcnt * C],
                        in_=src_ps[:].rearrange("p (g q) -> p g q", g=2)
                            [:, :, ls * C:(ls + lcnt) * C])

            # ---- pass 1: per-window segment sum -> A0s2h (w-outer, fp16) ----
            A0s2h = ctile('A0s2h', [EBLK, 2880], F16)
            for w in range(NWINC):
                psA0 = pp.tile([EBLK, GLC], F32, name='psA0', tag='seg')
                for g in range(2):
                    for bi in range(NBW):
                        blk = w * NBW + bi
                        nc.tensor.matmul(
                            out=psA0[:, g * LC:(g + 1) * LC],
                            lhsT=lhsT1[g][:, blk * EBLK:(blk + 1) * EBLK],
                            rhs=P[:, blk * LC:(blk + 1) * LC],
                            start=(bi == 0), stop=(bi == NBW - 1),
                            skip_group_check=True)
                dmaj_copy(A0s2h, psA0, w)

            # ---- window-merged radial-style transforms ----
            def merged_mm(pt_map, terms, emit_start=True, emit_stop=True, gouts=(0, 1)):
                """terms: list of (weight_field, src_tile).  For each psum
                region slice, accumulate  sum_gin  w[gout,d,gin].T @ src[gin]
                for every term.  pt_map: (gout, ti) -> psum tile."""
                for gout in gouts:
                    for ti, (width, slices) in enumerate(RAD_TILES):
                        pt = pt_map[(gout, ti)]
                        for (d, soff, wd, off) in slices:
                            seq = [(wf, st, gin) for wf, st in terms for gin in range(2)]
                            for j, (wf, st, gin) in enumerate(seq):
                                wcol = ((gout * 4 + d) * 2 + gin) * EBLK
                                nc.tensor.matmul(
                                    out=pt[:, off:off + wd],
                                    lhsT=s16[wf][:, wcol:wcol + EBLK],
                                    rhs=st[:, gin * NWINC * LC + soff:
                                           gin * NWINC * LC + soff + wd],
                                    start=(emit_start and j == 0),
                                    stop=(emit_stop and j == len(seq) - 1),
                                    skip_group_check=True)

            A_s2h = ctile('A_s2h', [EBLK, 2880], F16)
            for gout in range(2):
                pts = {(gout, ti): pr.tile([EBLK, wd], F32, name=f'psA_{ti}',
                                           tag=f'rad{ti}')
                       for ti, (wd, _) in enumerate(RAD_TILES)}
                merged_mm(pts, [('wbd_rad', A0s2h)], gouts=(gout,))
                for ti, (wdth, slices) in enumerate(RAD_TILES):
                    for (d, soff, wd, off) in slices:
                        nc.scalar.copy(
                            out=A_s2h[:, gout * 1440 + soff:gout * 1440 + soff + wd],
                            in_=pts[(gout, ti)][:, off:off + wd])

            # ---- symmetrize: Ain d-major -> Bs [p, (g, k5, w, c)] fp32 ----
            def symmetrize(Ain, tagpfx):
                Bs = ctile(tagpfx, [EBLK, 2 * NB * NWINC * C])
                for g in range(2):
                    gb = g * NWINC * LC
                    # nu=1 block: A degree-0 block is already (w, c)
                    nc.scalar.copy(
                        out=Bs[:, (g * NB) * NWINC * C:(g * NB + 1) * NWINC * C],
                        in_=Ain[:, gb:gb + NWINC * C])
                    for d, (ls, lcnt) in enumerate(GRP_SLICES):
                        Av = Ain[:, gb + DOFF[d]:gb + DOFF[d] + NWINC * lcnt * C] \
                            .rearrange("p (w l c) -> p w c l", w=NWINC, l=lcnt)
                        wm2 = ctile(f'{tagpfx}_wm{g}_{d}', [EBLK, NWINC * C * lcnt])
                        wv = wm2[:].rearrange("p (w c l) -> p w c l", w=NWINC, c=C)
                        nc.scalar.activation(out=wv, in_=Av, func=AF.Square)
                        if lcnt > 1:
                            TT(out=wv, in0=wv,
                               in1=s['multi_l'][:, ls:ls + lcnt]
                                   .unsqueeze(1).unsqueeze(2)
                                   .broadcast_to([EBLK, NWINC, C, lcnt]),
                               op=OP.mult)
                        ob = (g * NB + 1 + d) * NWINC * C
                        nc.vector.tensor_reduce(
                            Bs[:, ob:ob + NWINC * C]
                                .rearrange("p (w c) -> p w c", w=NWINC).unsqueeze(3),
                            wv, mybir.AxisListType.X, OP.add)
                return Bs

            B0s = symmetrize(A_s2h, 'B0')
            B0sh = ctile('B0sh', [EBLK, NWINC * 2 * NB * C], F16)
            nc.scalar.copy(out=B0sh[:], in_=B0s[:])

            # ---- chi (all windows at once): psC [32, (w, c)] ----
            psC = pp.tile([WIN, NWINC * C], F32, name='psC', tag='seg')
            first = True
            for g in range(2):
                for k in range(NB):
                    nc.tensor.matmul(
                        out=psC[:],
                        lhsT=s16['wbd_chi'][:, (g * NB + k) * WIN:(g * NB + k + 1) * WIN],
                        rhs=B0sh[:, (g * NB + k) * NWINC * C:
                                 (g * NB + k + 1) * NWINC * C],
                        start=first, stop=(g == 1 and k == NB - 1),
                        skip_group_check=True)
                    first = False
            chiS = ctile('chiS', [WIN, NWINC * C], F16)
            nc.scalar.copy(out=chiS[:], in_=psC[:])
            nc.sync.dma_start(out=outB[:, 0:NWINC * 2 * NB * C], in_=B0s[:])

            # ---- node table -> DRAM, AllGather (fp16, Shared output) ----
            F8 = mybir.dt.float8e4
            T_local = dp.tile([NWINC * WIN, TW], F8, name='T_local')
            T_full = dp.tile([NSLOT, TW], F8, name='T_full', addr_space="Shared")
            # stage [p, (w, g, l, c)] table layout in fp8 (gpsimd, off ACT)
            A_tab = ctile('A_tab', [EBLK, 2880], F8)
            for g in range(2):
                for d, (ls, lcnt) in enumerate(GRP_SLICES):
                    nc.gpsimd.tensor_copy(
                        out=A_tab[:].rearrange("p (w g q) -> p w g q",
                                               w=NWINC, g=2)
                            [:, :, g, ls * C:(ls + lcnt) * C],
                        in_=A_s2h[:, g * 1440 + DOFF[d]:
                                  g * 1440 + DOFF[d] + NWINC * lcnt * C]
                            .rearrange("p (w q) -> p w q", w=NWINC))
            for x in range(4):
                for g in range(2):
                    nc.sync.dma_start(
                        out=T_local[:, 0:RB * LC]
                            .rearrange("(w n) (b q) -> n w b q", n=WIN, b=RB)
                            [:, :, g * 4 + x],
                        in_=A_tab[x * WIN:(x + 1) * WIN, :]
                            .rearrange("p (w g q) -> p w g q", w=NWINC, g=2)
                            [:, :, g])
            nc.sync.dma_start(
                out=T_local[:, RB * LC:RB * LC + 2 * C].bitcast(F16)
                    .rearrange("(w n) c -> n w c", n=WIN),
                in_=chiS[:].rearrange("p (w c) -> p w c", w=NWINC))
            nc.gpsimd.collective_compute(
                "AllGather", mybir.AluOpType.bypass,
                replica_groups=[list(range(N_CORES))],
                ins=[T_local[:].opt()], outs=[T_full[:].opt()])

            # fr = (radf @ W_ar) * MP_NORM : [blk, b8] fp32
            frA = ctile('frA', [EBLK, NBLK * N_RBF])
            frB = ctile('frB', [EBLK, NBLK * N_RBF])
            frt = ctile('frt', [EBLK, NBLK * N_RBF])
            for r_ in range(N_RBF):
                radv = radf[:, r_ * NBLK:(r_ + 1) * NBLK].unsqueeze(2) \
                    .broadcast_to([EBLK, NBLK, N_RBF])
                warv = s['war_mp'][:, r_ * N_RBF:(r_ + 1) * N_RBF].unsqueeze(1) \
                    .broadcast_to([EBLK, NBLK, N_RBF])
                if r_ == 0:
                    TT(out=frA[:].rearrange("p (b k) -> p b k", k=N_RBF),
                       in0=radv, in1=warv, op=OP.mult)
                else:
                    TT(out=frt[:].rearrange("p (b k) -> p b k", k=N_RBF),
                       in0=radv, in1=warv, op=OP.mult)
                    src, dst = (frA, frB) if r_ % 2 == 1 else (frB, frA)
                    TT(out=dst[:], in0=src[:], in1=frt[:], op=OP.add)
            fr = frB   # 7 ping-pong adds end in frB
            fr_h = ctile('fr_h', [EBLK, NBLK * N_RBF], F16)
            nc.scalar.copy(out=fr_h[:], in_=fr[:])

            # lhsT_ar_g = fr-half (x) onehot : [blk, x4, n32] fp16
            lhsT_ar = []
            for g in range(2):
                lt = ctile(f'lhsT_ar_{g}', [EBLK, NBLK * EBLK],
                           mybir.dt.float8e4)
                TT(out=lt[:].rearrange("p (w b n) -> p w b n", b=4, n=WIN),
                   in0=fr_h[:].rearrange("p (w k) -> p w k", k=N_RBF)
                       [:, :, g * 4:(g + 1) * 4].unsqueeze(3)
                       .broadcast_to([EBLK, NBLK, 4, WIN]),
                   in1=onehot_h[:].rearrange("p (w n) -> p w n", w=NBLK).unsqueeze(2)
                       .broadcast_to([EBLK, NBLK, 4, WIN]),
                   op=OP.mult)
                lhsT_ar.append(lt)



            # ---- psMem for both gouts overlaps the AllGather ----
            Mem_s2 = ctile('Mem_s2', [EBLK, 2880])
            for gout in range(2):
                pmm = {(gout, ti): pr.tile([EBLK, wd], F32, name=f'psMm{gout}_{ti}',
                                           tag=f'rad{ti}')
                       for ti, (wd, _) in enumerate(RAD_TILES)}
                merged_mm(pmm, [('wbd_mem', A_s2h)], gouts=(gout,))
                for ti, (wdth, slices) in enumerate(RAD_TILES):
                    for (d, soff, wd, off) in slices:
                        nc.scalar.copy(
                            out=Mem_s2[:, gout * 1440 + soff:gout * 1440 + soff + wd],
                            in_=pmm[(gout, ti)][:, off:off + wd])

            # ---- pass 2 per-window: gather -> P2 -> psB0 -> psAr(Q) ----
            Ab0s2h = ctile('Ab0s2h', [EBLK, 2880], F16)
            Ar_s2 = ctile('Ar_s2', [EBLK, 2880])
            for w in range(NWINC):
                ags = []
                P2s = []
                for bi in range(NBW):
                    blk = w * NBW + bi
                    ag8 = gp.tile([EBLK, TW], F8, name='ag8', tag='ag8')
                    nc.gpsimd.indirect_dma_start(
                        out=ag8[:], out_offset=None, in_=T_full[:],
                        in_offset=bass.IndirectOffsetOnAxis(
                            ap=esrow_s[:, blk:blk + 1], axis=0))
                    if debug and w == 0 and bi == 0:
                        nc.sync.dma_start(out=dbg['ag0'][:], in_=ag8[:])
                    ags.append(ag8)
                    P2 = wp.tile([EBLK, LC], F16, name='P2', tag='P2')
                    TT(out=P2[:].rearrange("p (l c) -> p l c", c=C),
                       in0=P[:, blk * LC:(blk + 1) * LC].rearrange("p (l c) -> p l c", c=C),
                       in1=ag8[:, RB * LC:RB * LC + 2 * C].bitcast(F16)
                           .unsqueeze(1).broadcast_to([EBLK, NL, C]),
                       op=OP.mult)
                    P2s.append(P2)
                psB0 = pp.tile([EBLK, GLC], F32, name='psB0', tag='seg')
                for g in range(2):
                    for bi in range(NBW):
                        blk = w * NBW + bi
                        nc.tensor.matmul(
                            out=psB0[:, g * LC:(g + 1) * LC],
                            lhsT=lhsT1[g][:, blk * EBLK:(blk + 1) * EBLK],
                            rhs=P2s[bi][:],
                            start=(bi == 0), stop=(bi == NBW - 1),
                            skip_group_check=True)
                dmaj_copy(Ab0s2h, psB0, w)
                psAr = pp.tile([EBLK, GLC], F32, name='psAr', tag='ar')
                for g in range(2):
                    for x in range(4):
                        for bi in range(NBW):
                            blk = w * NBW + bi
                            nc.tensor.matmul(
                                out=psAr[x * WIN:(x + 1) * WIN, g * LC:(g + 1) * LC],
                                lhsT=lhsT_ar[g][:, blk * EBLK + x * WIN:
                                                blk * EBLK + (x + 1) * WIN],
                                rhs=ags[bi][:, (g * 4 + x) * LC:(g * 4 + x + 1) * LC],
                                start=(bi == 0), stop=(bi == NBW - 1),
                                skip_group_check=True,
                                tile_position=(0, x * WIN))
                dmaj_copy(Ar_s2, psAr, w)

            # ---- merged psAb, then combine with Ar and Mem ----
            Anew = ctile('Anew', [EBLK, 2880])
            for gout in range(2):
                pm = {(gout, ti): pr.tile([EBLK, wd], F32, name=f'psM{gout}_{ti}',
                                          tag=f'rad{ti}')
                      for ti, (wd, _) in enumerate(RAD_TILES)}
                merged_mm(pm, [('wbd_radmp', Ab0s2h)], gouts=(gout,))
                for ti, (wdth, slices) in enumerate(RAD_TILES):
                    for (d, soff, wd, off) in slices:
                        gs = slice(gout * 1440 + soff, gout * 1440 + soff + wd)
                        TT(out=Anew[:, gs],
                           in0=pm[(gout, ti)][:, off:off + wd],
                           in1=Ar_s2[:, gs], op=OP.add)
                        nc.gpsimd.tensor_tensor(
                            out=Anew[:, gs], in0=Anew[:, gs],
                            in1=Mem_s2[:, gs], op=OP.add)

            B1s = symmetrize(Anew, 'B1')

            if debug:
                nc.sync.dma_start(out=dbg['A_s'][:], in_=A_s2h[:])
                nc.sync.dma_start(out=dbg['A0s'][:], in_=A0s2h[:])
                nc.sync.dma_start(out=dbg['Anew'][:], in_=Anew[:])
                nc.sync.dma_start(out=dbg['chiS'][:], in_=chiS[:])
                nc.sync.dma_start(out=dbg['B0s'][:], in_=B0s[:])
                nc.sync.dma_start(out=dbg['P'][:], in_=P[:])
                nc.sync.dma_start(out=dbg['Ab0'][:], in_=Ab0s2h[:])
                nc.sync.dma_start(out=dbg['Ar'][:], in_=Ar_s2[:])
                nc.sync.dma_start(out=dbg['radf'][:], in_=radf[:])
                nc.sync.dma_start(out=dbg['fr'][:], in_=fr[:])
                nc.sync.dma_start(out=dbg['Tloc'][:], in_=T_local[:])

            # ---- output (B0 half already written right after pass 1) ----
            half = NWINC * 2 * NB * C
            nc.sync.dma_start(out=outB[:, half:2 * half], in_=B1s[:])

    nc.compile()
    return nc


_CACHE = {}


def kernel(**inputs) -> np.ndarray:
    return _kernel_impl(inputs, debug=False)[0]


def _kernel_impl(inputs, debug=False, trace=False):
    from concourse.bass_utils import run_bass_kernel_spmd

    packed, slot_of_node = _host_prep(inputs)

    key = 'ncdbg' if debug else 'nc'
    if key not in _CACHE:
        _CACHE[key] = _build_program(debug=debug)
    nc = _CACHE[key]

    in_maps = [dict(p) for p in packed]

    res = run_bass_kernel_spmd(nc, in_maps, core_ids=list(range(N_CORES)),
                               trace=trace)

    feats_slots = np.zeros((NSLOT, RB, NB, C, 2), np.float32)
    for ci in range(N_CORES):
        # outB [128, (t2, g2, k5, w8, c9)]; partition p = x*32 + n
        arr = res.results[ci]['outB'].reshape(4, WIN, 2, 2, NB, NWINC, C)
        # [x, n, t, g, k, w, c] -> [w, n, (g,x)=b, k, c, t]
        arr = np.transpose(arr, (5, 1, 3, 0, 4, 6, 2))
        feats_slots[ci * NWINC * WIN:(ci + 1) * NWINC * WIN] = \
            arr.reshape(NWINC * WIN, RB, NB, C, 2)
    return feats_slots[slot_of_node], res


if __name__ == '__main__':
    import pickle, os
    if os.path.exists('/tmp/inputs.pkl'):
        inputs = pickle.load(open('/tmp/inputs.pkl', 'rb'))
    else:
        import reference as Rf
        inputs = {k: np.asarray(v) for k, v in Rf.setup_inputs().items()}
        pickle.dump(inputs, open('/tmp/inputs.pkl', 'wb'))
    out = kernel(**inputs)
    print("kernel out", out.shape, out.dtype, float(np.abs(out).max()))
    if os.path.exists('/tmp/expected.npy'):
        exp = np.load('/tmp/expected.npy')
        err = np.abs(out - exp).max()
        print("max abs err vs expected:", err, "rel:", err / np.abs(exp).max())
